# revision 21
# baseline (speedup 1.0000x reference)
"""Causal self-attention (B=2, T=2048, C=1024, H=16, D=64) on 8 trn2 NeuronCores.

Sharding: core c = (batch b = c//4) x (head-group g = c%4; heads 4g..4g+3).
Tensor-parallel on heads for qkv_proj (column split) / out_proj (row split),
data-parallel on batch. Each core computes a full [C, T] partial of the
output projection for its batch; the host sums the 4 head-group partials
(bf16) per batch and transposes back to [T, C].

Device kernel (per core), all matmul operands bf16 (psum accumulate f32),
fused tq-chunk pipeline, per cq (512 tokens):
  1. QK(cq): W-stationary matmuls -> Q^T, K^T cols (+bias on DVE).
     V(cq): x^T-stationary matmuls -> V[t, d] tiles directly (no
     transpose); V bias folded in as a K=1 ones-row matmul.
  2. Attention(cq) in S^T layout per head pair: S^T tile = K_h @ Q_h^T
     (two heads packed in PE row strips, next tile's S issued before this
     tile's PV so the PE stays busy during exp), exp(x/8) on ACT, causal
     mask multiply on diagonal tiles (DVE h0 / gpsimd h1), U^T += V_aug^T
     @ P^T over tk tiles; psum row 64 accumulates the denominator l via a
     ones column in V_aug (filled once at start).
  3. Normalize (deferred past the other head-pair's t-loop so its DVE ops
     never block PE-critical work): l -> SBUF -> partition 0 -> broadcast
     -> approx-reciprocal -> scale U^T.
  4. Out-proj(cq-1): W_out-stationary matmuls on the previous chunk's
     normalized U^T; psum evacuation split DVE/ACT with fused bias.
"""

import sys

if "/opt/trn_rl_repo" not in sys.path:
    sys.path.insert(0, "/opt/trn_rl_repo")

import numpy as np

B, T, C = 2, 2048, 1024
H, D = 16, 64
HPC = 4            # heads per core
NC_ = HPC * D      # 256 qkv columns per core per projection
N_CORES = 8
PT = 128           # partition tile
TT = T // PT       # 16 t tiles
QC = 512           # tq chunk (moving free dim)
NQC = T // QC      # 4 tq chunks
KC = C // PT       # 8 contraction chunks for qkv proj
VA = D + 1         # v_aug live cols per head (64 V + 1 ones)
VAP = 80           # padded per-head stride in va tiles

_CACHE = {}


def _build_nc():
    import concourse.bacc as bacc
    import concourse.mybir as mybir
    import concourse.tile as tile
    from contextlib import ExitStack

    f32 = mybir.dt.float32
    bf16 = mybir.dt.bfloat16
    Act = mybir.ActivationFunctionType

    nc = bacc.Bacc("TRN2", target_bir_lowering=False, debug=False,
                   num_devices=N_CORES)

    xT_d = nc.dram_tensor("xT", [C, T], bf16, kind="ExternalInput").ap()
    wqkv_d = nc.dram_tensor("wqkv", [C, 3 * NC_], bf16, kind="ExternalInput").ap()
    bqkv_d = nc.dram_tensor("bqkv", [3 * NC_, 1], f32, kind="ExternalInput").ap()
    wout_d = nc.dram_tensor("wout", [NC_, C], bf16, kind="ExternalInput").ap()
    bout_d = nc.dram_tensor("bout", [C, 1], f32, kind="ExternalInput").ap()
    trimask_d = nc.dram_tensor("trimask", [PT, PT], bf16, kind="ExternalInput").ap()
    ident_d = nc.dram_tensor("ident", [PT, PT], bf16, kind="ExternalInput").ap()
    outT_d = nc.dram_tensor("outT", [C, T], bf16, kind="ExternalOutput").ap()

    with tile.TileContext(nc) as tc, ExitStack() as ctx:
        p_xt = ctx.enter_context(tc.tile_pool(name="xt", bufs=KC))
        p_wq = ctx.enter_context(tc.tile_pool(name="wq", bufs=KC))
        p_qk = ctx.enter_context(tc.tile_pool(name="qk", bufs=4))
        p_va = ctx.enter_context(tc.tile_pool(name="va", bufs=TT))
        p_vt = ctx.enter_context(tc.tile_pool(name="vt", bufs=4))
        p_wo = ctx.enter_context(tc.tile_pool(name="wo", bufs=2))
        p_un = ctx.enter_context(tc.tile_pool(name="un", bufs=2 * NQC))
        p_small = ctx.enter_context(tc.tile_pool(name="small", bufs=1))
        p_pt = ctx.enter_context(tc.tile_pool(name="ptile", bufs=4))
        p_norm = ctx.enter_context(tc.tile_pool(name="norm", bufs=4))
        p_out = ctx.enter_context(tc.tile_pool(name="outs", bufs=4))
        ps_1 = ctx.enter_context(tc.tile_pool(name="ps1", bufs=2, space="PSUM"))
        ps_2 = ctx.enter_context(tc.tile_pool(name="ps2", bufs=2, space="PSUM"))

        # ---- loads: two parallel DMA rails (sync: weights, gpsimd: x
        # chunk 0) + smalls on scalar so the first QK matmul starts ASAP;
        # x chunks 1..3 are prefetched inside the main loop ------------
        wq_sb = []
        for k in range(KC):
            w = p_wq.tile([PT, 3 * NC_], bf16, tag="wq")
            nc.sync.dma_start(w[:], wqkv_d[k * PT:(k + 1) * PT, :])
            wq_sb.append(w)

        xt_sb = [p_xt.tile([PT, T], bf16, tag="xt", name=f"xt{k}") for k in range(KC)]
        for k in range(KC):
            nc.gpsimd.dma_start(xt_sb[k][:, 0:QC],
                                xT_d[k * PT:(k + 1) * PT, 0:QC])
        for k in range(KC):
            nc.gpsimd.dma_start(xt_sb[k][:, QC:2 * QC],
                                xT_d[k * PT:(k + 1) * PT, QC:2 * QC])

        bq_sb = []
        for m in range(6):
            b = p_small.tile([PT, 1], f32, tag="bq", bufs=6)
            nc.scalar.dma_start(b[:], bqkv_d[m * PT:(m + 1) * PT, :])
            bq_sb.append(b)

        trimask = p_small.tile([PT, PT], bf16, tag="trimask")
        nc.scalar.dma_start(trimask[:], trimask_d[:])
        ident = p_small.tile([PT, PT], bf16, tag="ident")
        nc.scalar.dma_start(ident[:], ident_d[:])
        wo_sb = []
        for k in range(2):
            w = p_wo.tile([PT, C], bf16, tag="wo")
            nc.scalar.dma_start(w[:], wout_d[k * PT:(k + 1) * PT, :])
            wo_sb.append(w)
        bo_sb = []
        for e in range(C // PT):
            b = p_small.tile([PT, 1], f32, tag="bo", bufs=C // PT)
            nc.scalar.dma_start(b[:], bout_d[e * PT:(e + 1) * PT, :])
            bo_sb.append(b)

        qk_sb = [p_qk.tile([PT, T], bf16, tag="qk", name=f"qk{j}") for j in range(4)]
        va_sb = [p_va.tile([PT, HPC * VAP], bf16, tag="va", name=f"va{t}") for t in range(TT)]
        un_sb = [[p_un.tile([PT, QC], bf16, tag="un", name=f"un{j}_{c}")
                  for c in range(NQC)] for j in range(2)]

        # ones columns for the softmax row-sum, filled once up front
        # (V writes only the D v-columns per head)
        for t in range(TT):
            nc.gpsimd.memset(
                va_sb[t].rearrange("p (h v) -> p h v", v=VAP)[:, :, D:D + 1],
                1.0)

        def qkv_chunk(cq):
            # Q^T, K^T, V^T for tq cols [cq*QC, (cq+1)*QC); W-stationary,
            # V^T tiles flipped to V via PE transpose
            cs = slice(cq * QC, (cq + 1) * QC)
            for m in range(6):
                ps = ps_1.tile([PT, QC], f32, tag="sa")
                for k in range(KC):
                    nc.tensor.matmul(
                        ps[:],
                        wq_sb[k][:, m * PT:(m + 1) * PT],
                        xt_sb[k][:, cs],
                        start=(k == 0), stop=(k == KC - 1),
                    )
                if m < 4:
                    nc.vector.tensor_scalar_add(qk_sb[m][:, cs], ps[:], bq_sb[m])
                else:
                    h0, h1 = 2 * (m - 4), 2 * (m - 4) + 1
                    for q4 in range(4):
                        t = cq * 4 + q4
                        vtp = p_vt.tile([PT, PT], bf16, tag="vt")
                        nc.vector.tensor_scalar_add(
                            vtp[:], ps[:, q4 * PT:(q4 + 1) * PT], bq_sb[m])
                        pst = ps_1.tile([PT, PT], bf16, tag="sa")
                        nc.tensor.transpose(pst[:], vtp[:], ident[:])
                        nc.vector.tensor_copy(
                            va_sb[t][:, h0 * VAP:h0 * VAP + D], pst[:, 0:D])
                        nc.vector.tensor_copy(
                            va_sb[t][:, h1 * VAP:h1 * VAP + D], pst[:, D:2 * D])

        def attn_chunk(cq):
            # both head pairs interleaved per tk tile: each stream's exp
            # (ACT) hides the other stream's S/PV matmuls (PE)
            nts = 4 * cq + 4
            psu = [ps_2.tile([PT, 2 * QC], f32, tag="acc", name=f"psu{cq}_{j}")
                   for j in range(2)]

            def s_mm(j, t):
                p = t - 4 * cq
                s = max(p, 0) * PT
                psS = ps_1.tile([PT, 2 * QC], f32, tag="sa")
                tsl = slice(t * PT, (t + 1) * PT)
                qsl = slice(cq * QC + s, (cq + 1) * QC)
                nc.tensor.matmul(
                    psS[:, s:QC],
                    qk_sb[2 + j][0:D, tsl], qk_sb[j][0:D, qsl],
                    start=True, stop=True, tile_position=(0, 0),
                )
                nc.tensor.matmul(
                    psS[:, QC + s:2 * QC],
                    qk_sb[2 + j][D:PT, tsl], qk_sb[j][D:PT, qsl],
                    start=True, stop=True, tile_position=(D, 0),
                )
                return psS

            cur = [s_mm(0, 0), s_mm(1, 0)]
            for t in range(nts):
                p = t - 4 * cq      # >= 0 on diagonal-crossing tiles
                s = max(p, 0) * PT  # skip fully-masked leading columns
                pts = []
                for j in range(2):
                    pt = p_pt.tile([PT, 2 * QC], bf16, tag="pt")
                    pt3 = pt.rearrange("p (h w) -> p h w", h=2)
                    psS3 = cur[j].rearrange("p (h w) -> p h w", h=2)
                    nc.scalar.activation(pt3[:, :, s:QC], psS3[:, :, s:QC],
                                         Act.Exp, scale=0.125)
                    pts.append(pt)
                if t + 1 < nts:
                    nxt = [s_mm(0, t + 1), s_mm(1, t + 1)]
                for j in range(2):
                    if p >= 0:
                        nc.vector.tensor_mul(
                            pts[j][:, s:s + PT], pts[j][:, s:s + PT],
                            trimask[:])
                        nc.vector.tensor_mul(
                            pts[j][:, QC + s:QC + s + PT],
                            pts[j][:, QC + s:QC + s + PT], trimask[:])
                for j in range(2):
                    h0, h1 = 2 * j, 2 * j + 1
                    nc.tensor.matmul(
                        psu[j][0:VA, s:QC],
                        va_sb[t][:, h0 * VAP:h0 * VAP + VA], pts[j][:, s:QC],
                        start=(t == 0), stop=(t == nts - 1),
                    )
                    nc.tensor.matmul(
                        psu[j][0:VA, QC + s:2 * QC],
                        va_sb[t][:, h1 * VAP:h1 * VAP + VA],
                        pts[j][:, QC + s:2 * QC],
                        start=(t == 0), stop=(t == nts - 1),
                    )
                if t + 1 < nts:
                    cur = nxt
            return psu

        def normalize(cq, j, psu):
            # rows 0..63 = U^T, row 64 = l (both heads). l: psum row 64 ->
            # SBUF -> partition 0 (partition_broadcast ucode reads physical
            # partition 0) -> broadcast -> approx-reciprocal at offset 0
            # (reciprocal_approx_fast misbehaves at partition offset 64)
            rr = p_norm.tile([VA, 2 * QC], f32, tag="rr")
            nc.vector.tensor_copy(rr[D:VA, :], psu[D:VA, :])
            rl = p_norm.tile([D, 2 * QC], f32, tag="rl")
            nc.gpsimd.dma_start(rl[0:1, :], rr[D:VA, :])
            nc.gpsimd.partition_broadcast(rl[0:D, :], rl[0:1, :])
            rb = p_norm.tile([D, 2 * QC], f32, tag="rb")
            nc.vector.reciprocal_approx_fast(rb[:], rl[:])
            nc.vector.tensor_mul(un_sb[j][cq][0:D, :], psu[0:D, 0:QC],
                                 rb[:, 0:QC])
            ut = p_norm.tile([D, QC], bf16, tag="ut")
            nc.vector.tensor_mul(ut[:], psu[0:D, QC:2 * QC], rb[:, QC:2 * QC])
            nc.gpsimd.dma_start(un_sb[j][cq][D:PT, :], ut[:])

        def outproj_chunk(cq):
            cs = slice(cq * QC, (cq + 1) * QC)
            for ep in range(4):
                pp2 = ps_1.tile([PT, 2 * QC], f32, tag="sa")
                for half in range(2):
                    e = 2 * ep + half
                    for k in range(2):
                        nc.tensor.matmul(
                            pp2[:, half * QC:(half + 1) * QC],
                            wo_sb[k][:, e * PT:(e + 1) * PT],
                            un_sb[k][cq][:],
                            start=(k == 0), stop=(k == 1),
                        )
                ot = p_out.tile([PT, 2 * QC], bf16, tag="ot")
                for half in range(2):
                    e = 2 * ep + half
                    nc.vector.tensor_scalar_add(
                        ot[:, half * QC:(half + 1) * QC],
                        pp2[:, half * QC:(half + 1) * QC], bo_sb[e])
                    nc.sync.dma_start(
                        outT_d[e * PT:(e + 1) * PT, cs],
                        ot[:, half * QC:(half + 1) * QC])

        for cq in range(NQC):
            qkv_chunk(cq)
            if cq == 0:
                # prefetch the back half of x^T; lands during attention
                nx = slice(2 * QC, 4 * QC)
                for k in range(KC):
                    nc.sync.dma_start(xt_sb[k][:, nx],
                                      xT_d[k * PT:(k + 1) * PT, nx])
            psu = attn_chunk(cq)
            if cq > 0:
                outproj_chunk(cq - 1)
            normalize(cq, 0, psu[0])
            normalize(cq, 1, psu[1])
        outproj_chunk(NQC - 1)

    nc.compile()
    return nc


def _get_nc():
    if "nc" not in _CACHE:
        _CACHE["nc"] = _build_nc()
    return _CACHE["nc"]


def _make_in_maps(x, W_qkv, b_qkv, W_out, b_out):
    import ml_dtypes

    bf16 = ml_dtypes.bfloat16
    x = np.asarray(x, dtype=np.float32)
    W_qkv = np.asarray(W_qkv, dtype=np.float32)
    b_qkv = np.asarray(b_qkv, dtype=np.float32)
    W_out = np.asarray(W_out, dtype=np.float32)
    b_out = np.asarray(b_out, dtype=np.float32)

    i = np.arange(PT)[:, None]
    j = np.arange(PT)[None, :]
    trimask = (i <= j).astype(bf16)
    ident = np.eye(PT, dtype=bf16)

    in_maps = []
    for c in range(N_CORES):
        b, g = divmod(c, 4)
        gs = slice(g * NC_, (g + 1) * NC_)
        wqkv_c = np.ascontiguousarray(np.concatenate(
            [W_qkv[:, gs], W_qkv[:, C:][:, gs], W_qkv[:, 2 * C:][:, gs]],
            axis=1).astype(bf16))
        bqkv_c = np.ascontiguousarray(np.concatenate(
            [b_qkv[gs], b_qkv[C:][gs], b_qkv[2 * C:][gs]])[:, None])
        bout_c = (b_out if g == 0 else np.zeros_like(b_out))[:, None]
        in_maps.append({
            "xT": np.ascontiguousarray(x[b].T.astype(bf16)),
            "wqkv": wqkv_c,
            "bqkv": bqkv_c,
            "wout": np.ascontiguousarray(W_out[gs, :].astype(bf16)),
            "bout": np.ascontiguousarray(bout_c),
            "trimask": trimask,
            "ident": ident,
        })
    return in_maps


def _assemble(results):
    out = np.empty((B, T, C), dtype=np.float32)
    for b in range(B):
        acc = results[4 * b]["outT"].astype(np.float32)
        for g in range(1, 4):
            acc += results[4 * b + g]["outT"].astype(np.float32)
        out[b] = acc.T
    return out


def kernel(x, W_qkv, b_qkv, W_out, b_out):
    from concourse import bass_utils
    nc = _get_nc()
    in_maps = _make_in_maps(x, W_qkv, b_qkv, W_out, b_out)
    res = bass_utils.run_bass_kernel_spmd(nc, in_maps, core_ids=list(range(N_CORES)))
    return _assemble(res.results)
